# revision 2
# baseline (speedup 1.0000x reference)
"""FAPE loss kernel for Trainium2 (Bass/Tile), 8 NeuronCores.

Problem: B=8, N=1024.  reference computes, per batch b:
    R_i, t_i = backbone frames from (n, ca, c)          [N,3,3],[N,3]
    diff[i,j] = || R_i^T (pred_j - t_i) - R_i^T (true_j - t_i) ||
    per_pair  = min(diff,10) + 0.5*(diff - min(diff,10)) = 0.5*(diff + min(diff,10))
    out = sum_b sum_ij m_i m_j per_pair / (sum(m) + 1e-8)

Key reassociation (exact, no orthonormality assumption):
    R_i^T (pred_j - t_i) - R_i^T (true_j - t_i) = R_i^T d_j,  d_j = pred_j - true_j
    diff^2[i,j] = d_j^T (R_i R_i^T) d_j = sum_k q_k[j] * w_k[i]   (K=6)
  with q[j] = [d0^2, d1^2, d2^2, 2*d0d1, 2*d1d2, 2*d2d0] (masked by m_j)
       w[i] = [G00, G11, G22, G01, G12, G20], G = R_i R_i^T (masked by m_i)

Pairwise O(N^2) part per j-tile of 128:
  - TensorEngine matmul, K=18 bf16 hi/lo split (q = qh+ql, w = wh+wl bf16;
    rows [qh,qh,ql] x [wh,wl,wh] -> q.w exact up to ~2^-18) -> normsq PSUM
  - ACT sqrt PSUM->SBUF bf16
  - two DVE tensor_scalar passes at 4x bf16 rate: min(diff,10) and copy,
    each with accum_out giving the per-row sums
Per-core output is the [128, 8+8] accumulator sheet (sum_i per j-row for
diff and clamped); host sums sheets from the 8 cores and normalizes.

Frame-build is sqrt-free (all deviations ~1e-7 relative, far below the
~2^-18 matmul and bf16-diff rounding this kernel already carries): since
x, y, z only enter G quadratically, their normalizations become reciprocal
scale factors on raw cross-product Grams,
    G = ux ux^T/ns_x + zr zr^T/ns_z + yr yr^T/(ns_x ns_z),
with ux = c-ca, zr = cross(ux, n-ca), yr = cross(zr, ux) — a pure-DVE
chain (no cross-engine sqrt round-trips on the critical path).

Sharding: batch-parallel, one batch per core (spec hint allows B data-parallel).
"""

import numpy as np

P = 128          # partitions
T = 8            # j = 8*p + t  (p-major; any index bijection works for the sum)
N = 1024
B = 8
NCORES = 8
NACC = 2 * (T + 1)  # accumulator columns (iter 0 split per chunk)

_cache: dict = {}


def _build_nc(diff_dtype="bf16", reps=0, prep_only=False):
    """Emit the single-core BIR module (same NEFF runs SPMD on all 8 cores)."""
    from contextlib import ExitStack

    import concourse.bacc as bacc
    import concourse.mybir as mybir
    import concourse.tile as tile
    from concourse import masks
    from concourse._compat import axon_active

    f32 = mybir.dt.float32
    bf16 = mybir.dt.bfloat16
    d_dt = bf16 if diff_dtype == "bf16" else f32
    Alu = mybir.AluOpType
    Act = mybir.ActivationFunctionType
    AxX = mybir.AxisListType.X

    nc = bacc.Bacc(
        "TRN2",
        target_bir_lowering=False,
        debug=not axon_active(),
        num_devices=NCORES,
    )

    # One concatenated input: cols [n(3) c(3) ca(3) pred(3) true(3) mask(1)]
    d_all = nc.dram_tensor("all_in", [N, 16], f32, kind="ExternalInput")
    d_out = nc.dram_tensor("out_acc", [P, NACC], f32, kind="ExternalOutput")

    with tile.TileContext(nc) as tc, ExitStack() as ctx:
        sb = ctx.enter_context(tc.tile_pool(name="sb", bufs=1))
        ps_t = ctx.enter_context(tc.tile_pool(name="ps_t", bufs=2, space="PSUM"))
        ps_ns = ctx.enter_context(tc.tile_pool(name="ps_ns", bufs=2, space="PSUM"))
        dpool = ctx.enter_context(tc.tile_pool(name="dpool", bufs=3))
        spool = ctx.enter_context(tc.tile_pool(name="spool", bufs=3))

        # ---- ACT table warmup: force the sqrt set load early (overlaps DMA)
        warm = sb.tile([1, 2], f32)
        nc.vector.memset(warm[:], 1.0)
        nc.scalar.activation(warm[:, 1:2], warm[:, 0:1], Act.Sqrt)

        # ---- ONE input DMA: [1024,16] -> [128, 8, 16], j = 8*p + t.
        # Fully contiguous in DRAM, 512B per partition.  Issued first.
        stg = sb.tile([P, T, 16], f32)
        nc.sync.dma_start(stg[:], d_all.ap().rearrange("(p t) c -> p t c", p=P))

        ident = sb.tile([P, P], bf16)
        masks.make_identity(nc, ident[:])

        rep_ctx = tc.For_i(0, reps, 1) if reps else None
        if rep_ctx is not None:
            rep_ctx.__enter__()
        t_nc2 = stg[:, :, 0:6].rearrange("p t (a c) -> p t a c", a=2)
        t_ca1 = stg[:, :, 6:9]
        t_pred = stg[:, :, 9:12]
        t_true = stg[:, :, 12:15]
        mask_bc6 = stg[:, :, 15:16].broadcast_to([P, T, 6])

        # ---- helpers ------------------------------------------------------
        def replicate(vec, name, eng=None):
            """[128,8,3] view -> [128,8,6] with r2[:, t, c] = vec[:, t, c % 3]."""
            r2 = sb.tile([P, T, 6], f32, tag=name)
            (eng or nc.vector).tensor_copy(
                r2[:].rearrange("p t (r c) -> p t r c", r=2),
                vec.unsqueeze(2).broadcast_to([P, T, 2, 3]),
            )
            return r2

        def cross(a2, b2, name, out=None):
            m1 = sb.tile([P, T, 3], f32, tag=f"{name}_m1")
            m2 = sb.tile([P, T, 3], f32, tag=f"{name}_m2")
            if out is None:
                out = sb.tile([P, T, 3], f32, tag=name)
            nc.vector.tensor_tensor(m1[:], a2[:, :, 1:4], b2[:, :, 2:5], Alu.mult)
            nc.vector.tensor_tensor(m2[:], a2[:, :, 2:5], b2[:, :, 1:4], Alu.mult)
            nc.vector.tensor_tensor(out[:], m1[:], m2[:], Alu.subtract)
            return out

        def hi_lo_stack(src, layout, name, eng=None):
            """src f32 [128,8,6] -> bf16 [128,8,18] stacked per `layout`
            ('h' = bf16 rounding of src, 'l' = residual src - hi)."""
            eng = eng or nc.vector
            out = sb.tile([P, T, 18], bf16, tag=name)
            hi_slot = layout.index('h') * 6
            eng.tensor_copy(out[:, :, hi_slot:hi_slot + 6], src[:])
            for g, kind in enumerate(layout):
                if g * 6 == hi_slot:
                    continue
                sl = out[:, :, g * 6:(g + 1) * 6]
                if kind == 'h':
                    eng.tensor_copy(sl, out[:, :, hi_slot:hi_slot + 6])
                else:
                    eng.tensor_tensor(
                        sl, src[:], out[:, :, hi_slot:hi_slot + 6], Alu.subtract)
            return out

        def hi_lo_stack_halves(src, layout, name, eng=None):
            """hi_lo_stack but emitted per t-half so the PE transposes of
            half 0 can start while half 1 is still stacking."""
            eng = eng or nc.vector
            out = sb.tile([P, T, 18], bf16, tag=name)
            hi_slot = layout.index('h') * 6
            for h in range(2):
                ts = slice(h * (T // 2), (h + 1) * (T // 2))
                eng.tensor_copy(out[:, ts, hi_slot:hi_slot + 6], src[:, ts])
                for g, kind in enumerate(layout):
                    if g * 6 == hi_slot:
                        continue
                    sl = out[:, ts, g * 6:(g + 1) * 6]
                    if kind == 'h':
                        eng.tensor_copy(sl, out[:, ts, hi_slot:hi_slot + 6])
                    else:
                        eng.tensor_tensor(sl, src[:, ts],
                                          out[:, ts, hi_slot:hi_slot + 6],
                                          Alu.subtract)
            return out

        # ---- frames -> Gram, sqrt-free.  The reference builds
        #   x = normalize(c-ca), v = normalize(n-ca), z = normalize(x X v),
        #   y = normalize(z X x),  G = xx^T + yy^T + zz^T.
        # Positive scale factors pass through cross products and cancel in
        # the direction, so with ux = c-ca, zr = cross(ux, n-ca),
        # yr = cross(zr, ux):
        #   xx^T = ux ux^T / ns_x,  zz^T = zr zr^T / ns_z,
        #   yy^T = yr yr^T / (ns_z * ns_x)        (||y_ref|| = 1 to ~1e-7)
        # with ns_* plain sums of squares -> the whole chain is DVE-only
        # (reciprocal instead of 1/(sqrt+eps); deviation ~1e-8 relative).
        u2 = sb.tile([P, 2, T, 3], f32)
        nc.vector.tensor_tensor(
            u2[:].rearrange("p a t c -> p t a c"), t_nc2,
            t_ca1.unsqueeze(2).broadcast_to([P, T, 2, 3]),
            Alu.subtract)
        # combined replicate of both u vectors: [128,2,8,3] -> [128,2,8,6]
        uu2 = sb.tile([P, 2, T, 6], f32)
        nc.vector.tensor_copy(
            uu2[:].rearrange("p a t (r c) -> p a t r c", r=2),
            u2[:].unsqueeze(3).broadcast_to([P, 2, T, 2, 3]),
        )
        uv2 = uu2[:, 0]
        ux2 = uu2[:, 1]
        zr = cross(ux2, uv2, "zr")
        zr2 = replicate(zr[:], "zr2")
        yr = cross(zr2, ux2, "yr")
        yr2 = replicate(yr[:], "yr2")
        sq2 = sb.tile([P, 2, T, 3], f32)
        nc.vector.tensor_tensor(sq2[:, 0], u2[:, 1], u2[:, 1], Alu.mult)
        nc.vector.tensor_tensor(sq2[:, 1], zr[:], zr[:], Alu.mult)
        ns2 = sb.tile([P, 2, T], f32)
        nc.vector.tensor_reduce(ns2[:], sq2[:], AxX, Alu.add)
        iv2 = sb.tile([P, 2, T], f32)
        nc.vector.reciprocal(iv2[:], ns2[:])
        # fold the i-mask into the scale factors (mask^2 = mask for 0/1
        # masks): every Gram term then carries it, so no separate w mask op.
        iv2m = sb.tile([P, 2, T], f32)
        nc.vector.tensor_tensor(
            iv2m[:], iv2[:],
            stg[:, :, 15].unsqueeze(1).broadcast_to([P, 2, T]), Alu.mult)
        ivzx = sb.tile([P, T], f32)
        nc.vector.tensor_tensor(ivzx[:], iv2m[:, 0], iv2m[:, 1], Alu.mult)

        # ---- w[i]: Gram of R_i, mask-folded.  P* = [diag(3) | offdiag(3)]
        def products(r2, name):
            out = sb.tile([P, T, 6], f32, tag=name)
            nc.vector.tensor_tensor(out[:, :, 0:3], r2[:, :, 0:3], r2[:, :, 0:3],
                                    Alu.mult)
            nc.vector.tensor_tensor(out[:, :, 3:6], r2[:, :, 0:3], r2[:, :, 1:4],
                                    Alu.mult)
            return out

        px = products(ux2, "px")
        pz = products(zr2, "pz")
        py = products(yr2, "py")
        bc6 = lambda v: v.unsqueeze(2).broadcast_to([P, T, 6])
        mx = sb.tile([P, T, 6], f32)
        mz = sb.tile([P, T, 6], f32)
        my = sb.tile([P, T, 6], f32)
        s1 = sb.tile([P, T, 6], f32)
        w_all = sb.tile([P, T, 6], f32)
        nc.vector.tensor_tensor(mx[:], px[:], bc6(iv2m[:, 0]), Alu.mult)
        nc.vector.tensor_tensor(mz[:], pz[:], bc6(iv2m[:, 1]), Alu.mult)
        nc.vector.tensor_tensor(my[:], py[:], bc6(ivzx[:]), Alu.mult)
        nc.vector.tensor_tensor(s1[:], mx[:], mz[:], Alu.add)
        nc.vector.tensor_tensor(w_all[:], s1[:], my[:], Alu.add)
        w18 = hi_lo_stack_halves(w_all, "hlh", "w18")  # rows [wh, wl, wh]

        # ---- q[j] path, entirely on Pool (gpsimd): keeps the DVE queue free
        # for the frame chain that gates the main loop.
        dd = sb.tile([P, T, 3], f32)
        nc.gpsimd.tensor_tensor(dd[:], t_pred, t_true, Alu.subtract)
        d2 = replicate(dd[:], "d2", eng=nc.gpsimd)
        q_all = sb.tile([P, T, 6], f32)
        nc.gpsimd.tensor_tensor(q_all[:, :, 0:3], dd[:], dd[:], Alu.mult)
        qc = sb.tile([P, T, 3], f32)
        nc.gpsimd.tensor_tensor(qc[:], d2[:, :, 0:3], d2[:, :, 1:4], Alu.mult)
        nc.gpsimd.tensor_tensor(q_all[:, :, 3:6], qc[:], qc[:], Alu.add)
        q_m = sb.tile([P, T, 6], f32)
        nc.gpsimd.tensor_tensor(q_m[:], q_all[:], mask_bc6, Alu.mult)
        q18 = hi_lo_stack(q_m, "hhl", "q18", eng=nc.gpsimd)  # rows [qh, qh, ql]

        # q transposes -> qT [18, 1024] bf16 (copies on ACT: keeps the DVE
        # queue free for the frame chain)
        qT = sb.tile([18, N], bf16)
        for half in range(2):
            pst = ps_t.tile([18, 4, P], bf16, tag="pst")
            for tt in range(4):
                t = half * 4 + tt
                nc.tensor.transpose(pst[:, tt, :], q18[:, t, :], ident[:])
            nc.scalar.copy(
                qT[:, half * 512:(half + 1) * 512],
                pst[:].rearrange("k f p -> k (f p)"),
            )

        # w transposes -> wT [18, 1024] bf16 (i' = transpose order; any
        # bijection of i is fine for the sum since mask is already folded in)
        wT = [sb.tile([18, 512], bf16, tag=f"wT{h}", name=f"wT{h}") for h in range(2)]
        for half in range(2):
            psw = ps_t.tile([18, 4, P], bf16, tag="pst")
            for tt in range(4):
                t = half * 4 + tt
                nc.tensor.transpose(psw[:, tt, :], w18[:, t, :], ident[:])
            nc.vector.tensor_copy(
                wT[half][:], psw[:].rearrange("k f p -> k (f p)"))

        # ---- main O(N^2) loop: per j-tile of 128, all 1024 i.  Iteration 0
        # runs per 512-chunk so its first sqrt starts as soon as wT[0] is
        # copied, without waiting for wT[1]'s transposes.  acc columns:
        # [d_t0c0, d_t0c1, d_t1..d_t7 | c_t0c0, c_t0c1, c_t1..c_t7];
        # the host just sums the whole sheet.
        acc = sb.tile([P, NACC], f32)
        HA = NACC // 2
        for t in range(1 if prep_only else T):
            nst = ps_ns.tile([P, N], f32, tag="nst")
            lhs = qT[:, t * P:(t + 1) * P]
            if t == 0 and not prep_only:
                for cch in range(2):
                    sl = slice(cch * 512, (cch + 1) * 512)
                    nc.tensor.matmul(nst[:, sl], lhs, wT[cch][:],
                                     start=True, stop=True)
                    dfh = dpool.tile([P, 512], d_dt, tag=f"dfh{cch}")
                    nc.scalar.activation(dfh[:], nst[:, sl], Act.Sqrt)
                    sch = spool.tile([P, 512], d_dt, tag=f"sch{cch}")
                    nc.vector.tensor_scalar(
                        sch[:], dfh[:], 10.0, 0.0, Alu.min, Alu.add,
                        accum_out=acc[:, HA + cch:HA + cch + 1])
                    sch2 = spool.tile([P, 512], d_dt, tag=f"sch2{cch}")
                    nc.vector.tensor_scalar(
                        sch2[:], dfh[:], 1.0, 0.0, Alu.mult, Alu.add,
                        accum_out=acc[:, cch:cch + 1])
                continue
            nc.tensor.matmul(nst[:, 0:512], lhs, wT[0][:], start=True, stop=True)
            nc.tensor.matmul(nst[:, 512:N], lhs, wT[1][:], start=True, stop=True)
            dft = dpool.tile([P, N], d_dt, tag="dft")
            nc.scalar.activation(dft[:], nst[:], Act.Sqrt)
            scr = spool.tile([P, N], d_dt, tag="scr")
            nc.vector.tensor_scalar(scr[:], dft[:], 10.0, 0.0, Alu.min, Alu.add,
                                    accum_out=acc[:, HA + 1 + t:HA + 2 + t])
            scr2 = spool.tile([P, N], d_dt, tag="scr2")
            nc.vector.tensor_scalar(scr2[:], dft[:], 1.0, 0.0, Alu.mult, Alu.add,
                                    accum_out=acc[:, 1 + t:2 + t])

        # ---- per-core partial sums out; host reduces the 128x16 sheet.
        # Bulk columns go out while the last iteration still runs.
        HB = NACC // 2
        oview = d_out.ap().rearrange("p (h t) -> p h t", h=2)
        aview = acc[:].rearrange("p (h t) -> p h t", h=2)
        nc.sync.dma_start(oview[:, :, 0:HB - 1], aview[:, :, 0:HB - 1])
        nc.sync.dma_start(oview[:, :, HB - 1:HB], aview[:, :, HB - 1:HB])

        if rep_ctx is not None:
            rep_ctx.__exit__(None, None, None)

    nc.compile()
    return nc


def _get_nc():
    if "nc" not in _cache:
        _cache["nc"] = _build_nc()
    return _cache["nc"]


def make_inmaps(n, ca, c, pred_pos, true_pos, mask):
    allc = np.concatenate(
        [np.asarray(n, np.float32), np.asarray(c, np.float32),
         np.asarray(ca, np.float32), np.asarray(pred_pos, np.float32),
         np.asarray(true_pos, np.float32),
         mask.astype(np.float32)[..., None]], axis=-1)
    allc = np.pad(allc, [(0, 0), (0, 0), (0, 16 - allc.shape[-1])])
    return [{"all_in": np.ascontiguousarray(allc[b])} for b in range(B)]


def kernel(n, ca, c, pred_pos, true_pos, mask) -> np.ndarray:
    from concourse.bass_utils import run_bass_kernel_spmd

    nc = _get_nc()
    in_maps = make_inmaps(n, ca, c, pred_pos, true_pos, mask)
    res = run_bass_kernel_spmd(nc, in_maps, core_ids=list(range(NCORES)))
    total = float(sum(r["out_acc"].astype(np.float64).sum() for r in res.results))
    denom = float(mask.sum()) + 1e-8
    return np.asarray(0.5 * total / denom, dtype=np.float32)



# revision 3
# speedup vs baseline: 2.1614x; 2.1614x over previous
"""FAPE loss kernel for Trainium2 (Bass/Tile), 8 NeuronCores.

Problem: B=8, N=1024.  reference computes, per batch b:
    R_i, t_i = backbone frames from (n, ca, c)          [N,3,3],[N,3]
    diff[i,j] = || R_i^T (pred_j - t_i) - R_i^T (true_j - t_i) ||
    per_pair  = min(diff,10) + 0.5*(diff - min(diff,10)) = 0.5*(diff + min(diff,10))
    out = sum_b sum_ij m_i m_j per_pair / (sum(m) + 1e-8)

Key identity: both pred and true are expressed in the SAME frame i, so
    R_i^T (pred_j - t_i) - R_i^T (true_j - t_i) = R_i^T d_j,  d_j = pred_j - true_j
and R_i is orthonormal by construction (x, y, z mutually orthogonal unit
vectors from normalized cross products), hence
    diff[i,j] = ||R_i^T d_j|| = ||d_j||            (independent of i!)
up to the 1e-8 normalize-eps and f32 rounding (~1e-7 relative, verified
6e-7 end-to-end vs the jax reference; tolerance is 2e-2).  The O(N^2)
pairwise reduction therefore factorizes exactly:
    sum_ij m_i m_j f(||d_j||) = (sum_i m_i) * (sum_j m_j f(||d_j||))
leaving O(N) device work per batch: one masked norm + clamp + row-sum.

Per-core body (one batch per core, j = 8*p + t, 7 instructions):
    d    = pred - true                       [128,8,3]  DVE
    sq   = d*d                               [128,8,3]  DVE
    nsq  = reduce_X(sq)                      [128,8]    DVE
    nsqm = nsq * mask   (m in {0,1}: sqrt(m*nsq) = m*diff)       DVE
    diff = sqrt(nsqm), accum_out -> acc[:,1]  (sum_t m*diff)     ACT
    clp  = min(diff,10), accum_out -> acc[:,0] (sum_t m*clamped) DVE
    DMA acc [128,2] -> HBM
Host sums the 128x2 sheets (f64), applies the (sum_i m_i) factor per
batch, and normalizes by sum(m) + 1e-8.

Sharding: batch-parallel, one batch per core (spec hint allows B data-parallel).
"""

import numpy as np

P = 128          # partitions
T = 8            # j = 8*p + t  (p-major; any index bijection works for the sum)
N = 1024
B = 8
NCORES = 8

_cache: dict = {}


def _build_nc(reps=0, prep_only=False):
    """Emit the single-core BIR module (same NEFF runs SPMD on all 8 cores)."""
    from contextlib import ExitStack

    import concourse.bacc as bacc
    import concourse.mybir as mybir
    import concourse.tile as tile
    from concourse._compat import axon_active

    f32 = mybir.dt.float32
    Alu = mybir.AluOpType
    Act = mybir.ActivationFunctionType
    AxX = mybir.AxisListType.X

    nc = bacc.Bacc(
        "TRN2",
        target_bir_lowering=False,
        debug=not axon_active(),
        num_devices=NCORES,
    )

    # One concatenated input: cols [pred(3) true(3) mask(1) pad(1)]
    d_all = nc.dram_tensor("all_in", [N, 8], f32, kind="ExternalInput")
    d_out = nc.dram_tensor("out_acc", [P, 2], f32, kind="ExternalOutput")

    with tile.TileContext(nc) as tc, ExitStack() as ctx:
        sb = ctx.enter_context(tc.tile_pool(name="sb", bufs=1))
        wpool = ctx.enter_context(tc.tile_pool(name="wpool", bufs=3))
        apool = ctx.enter_context(tc.tile_pool(name="apool", bufs=3))

        # ---- ACT table warmup: force the sqrt set load early (overlaps DMA)
        warm = sb.tile([1, 2], f32)
        nc.vector.memset(warm[:], 1.0)
        nc.scalar.activation(warm[:, 1:2], warm[:, 0:1], Act.Sqrt)

        # ---- ONE input DMA: [1024,8] -> [128, 8, 8], j = 8*p + t.
        # Fully contiguous in DRAM, 256B per partition.
        stg = sb.tile([P, T, 8], f32)
        nc.sync.dma_start(stg[:], d_all.ap().rearrange("(p t) c -> p t c", p=P))

        rep_ctx = tc.For_i(0, reps, 1) if reps else None
        if rep_ctx is not None:
            rep_ctx.__enter__()

        t_pred = stg[:, :, 0:3]
        t_true = stg[:, :, 3:6]
        t_mask = stg[:, :, 6]

        d = wpool.tile([P, T, 3], f32, tag="d")
        nc.vector.tensor_tensor(d[:], t_pred, t_true, Alu.subtract)
        sq = wpool.tile([P, T, 3], f32, tag="sq")
        nc.vector.tensor_tensor(sq[:], d[:], d[:], Alu.mult)
        nsq = wpool.tile([P, T], f32, tag="nsq")
        nc.vector.tensor_reduce(nsq[:], sq[:], AxX, Alu.add)
        nsqm = wpool.tile([P, T], f32, tag="nsqm")
        nc.vector.tensor_tensor(nsqm[:], nsq[:], t_mask, Alu.mult)

        acc = apool.tile([P, 2], f32, tag="acc")
        diff = wpool.tile([P, T], f32, tag="diff")
        nc.scalar.activation(diff[:], nsqm[:], Act.Sqrt,
                             accum_out=acc[:, 1:2])
        clp = wpool.tile([P, T], f32, tag="clp")
        nc.vector.tensor_scalar(clp[:], diff[:], 10.0, 0.0, Alu.min, Alu.add,
                                accum_out=acc[:, 0:1])

        nc.sync.dma_start(d_out.ap(), acc[:])

        if rep_ctx is not None:
            rep_ctx.__exit__(None, None, None)

    nc.compile()
    return nc


def _get_nc():
    if "nc" not in _cache:
        _cache["nc"] = _build_nc()
    return _cache["nc"]


def make_inmaps(n, ca, c, pred_pos, true_pos, mask):
    allc = np.empty((B, N, 8), np.float32)
    allc[:, :, 0:3] = np.asarray(pred_pos, np.float32)
    allc[:, :, 3:6] = np.asarray(true_pos, np.float32)
    allc[:, :, 6] = np.asarray(mask).astype(np.float32)
    allc[:, :, 7] = 0.0
    return [{"all_in": allc[b]} for b in range(B)]


def kernel(n, ca, c, pred_pos, true_pos, mask) -> np.ndarray:
    from concourse.bass_utils import run_bass_kernel_spmd

    nc = _get_nc()
    in_maps = make_inmaps(n, ca, c, pred_pos, true_pos, mask)
    res = run_bass_kernel_spmd(nc, in_maps, core_ids=list(range(NCORES)))
    m = np.asarray(mask).astype(np.float64)
    c_b = m.sum(axis=1)                      # per-batch masked-residue count
    total = 0.0
    for b in range(B):
        sheet = res.results[b]["out_acc"].astype(np.float64)
        total += c_b[b] * 0.5 * sheet.sum()
    return np.asarray(total / (m.sum() + 1e-8), dtype=np.float32)


# revision 8
# speedup vs baseline: 4.8925x; 2.2636x over previous
"""FAPE loss kernel for Trainium2 (Bass/Tile), 8 NeuronCores.

Problem: B=8, N=1024.  reference computes, per batch b:
    R_i, t_i = backbone frames from (n, ca, c)          [N,3,3],[N,3]
    diff[i,j] = || R_i^T (pred_j - t_i) - R_i^T (true_j - t_i) ||
    per_pair  = min(diff,10) + 0.5*(diff - min(diff,10)) = 0.5*(diff + min(diff,10))
    out = sum_b sum_ij m_i m_j per_pair / (sum(m) + 1e-8)

Key identity: both pred and true are expressed in the SAME frame i, so
    R_i^T (pred_j - t_i) - R_i^T (true_j - t_i) = R_i^T d_j,  d_j = pred_j - true_j
and R_i is orthonormal by construction (x, y, z mutually orthogonal unit
vectors from normalized cross products), hence
    diff[i,j] = ||R_i^T d_j|| = ||d_j||            (independent of i!)
up to the 1e-8 normalize-eps and f32 rounding (~1e-7 relative, verified
6e-7 end-to-end vs the jax reference; tolerance is 2e-2).  The O(N^2)
pairwise reduction therefore factorizes exactly:
    sum_ij m_i m_j f(||d_j||) = (sum_i m_i) * (sum_j m_j f(||d_j||))
leaving O(N) device work per batch: one masked norm + clamp + row-sum.

Per-core body (one batch per core, j = 8*p + t, 9 instructions).  The
host only needs the single scalar sum_j m_j*(diff_j + min(diff_j,10)),
so clamp+add fuse into one scalar_tensor_tensor and the whole thing
funnels through one PE partition-reduce:
    d    = pred - true                       [128,8,3]  DVE
    sq   = d*d                               [128,8,3]  Pool
    nsq  = reduce_X(sq)                      [128,8]    DVE
    nsqm = nsq * mask   (m in {0,1}: sqrt(m*nsq) = m*diff)       DVE
    dm   = sqrt(nsqm)           (= m*diff)   [128,8]    ACT
    s    = (dm min 10) + dm                  [128,8]    DVE
    ps   = ones[128,1]^T @ s[128,8] -> [1,8]  (partition sum)    PE
    sbo  = reduce_X(ps) -> [1,1]                                 DVE
    DMA sbo [1,1] -> HBM  (single 4-byte descriptor, Pool-queue
    issued: DMA_SEQ_TIME is ~25ns on Pool vs ~565ns on SP)
Host scales by 0.5 and the (sum_i m_i) factor per batch (f64), and
normalizes by sum(m) + 1e-8.

The bench loop (reps>0) unrolls UNROLL bodies per For_i iteration: the
For_i back-edge runs an all-engine barrier (~1.3us) that would otherwise
dwarf the body; unrolling amortizes it and lets consecutive bodies
pipeline through the tile-pool buffer rotation, so the measured slope is
the true steady-state per-body cost.

Sharding: batch-parallel, one batch per core (spec hint allows B data-parallel).
"""

import numpy as np

P = 128          # partitions
T = 8            # j = 8*p + t  (p-major; any index bijection works for the sum)
N = 1024
B = 8
NCORES = 8
UNROLL = 10

_cache: dict = {}


def _build_nc(reps=0, prep_only=False):
    """Emit the single-core BIR module (same NEFF runs SPMD on all 8 cores)."""
    from contextlib import ExitStack

    import concourse.bacc as bacc
    import concourse.mybir as mybir
    import concourse.tile as tile
    from concourse._compat import axon_active

    f32 = mybir.dt.float32
    Alu = mybir.AluOpType
    Act = mybir.ActivationFunctionType
    AxX = mybir.AxisListType.X

    nc = bacc.Bacc(
        "TRN2",
        target_bir_lowering=False,
        debug=not axon_active(),
        num_devices=NCORES,
    )

    # One concatenated input: cols [pred(3) true(3) mask(1) pad(1)]
    d_all = nc.dram_tensor("all_in", [N, 8], f32, kind="ExternalInput")
    d_out = nc.dram_tensor("out_acc", [1, 1], f32, kind="ExternalOutput")

    with tile.TileContext(nc) as tc, ExitStack() as ctx:
        sb = ctx.enter_context(tc.tile_pool(name="sb", bufs=1))
        wpool = ctx.enter_context(tc.tile_pool(name="wpool", bufs=3))
        opool = ctx.enter_context(tc.tile_pool(name="opool", bufs=3))
        pspool = ctx.enter_context(tc.tile_pool(name="pspool", bufs=4,
                                                space="PSUM"))

        # ---- ACT table warmup: force the sqrt set load early (overlaps DMA)
        warm = sb.tile([1, 2], f32)
        nc.vector.memset(warm[:], 1.0)
        nc.scalar.activation(warm[:, 1:2], warm[:, 0:1], Act.Sqrt)

        ones = sb.tile([P, 1], f32)
        nc.vector.memset(ones[:], 1.0)

        # ---- ONE input DMA: [1024,8] -> [128, 8, 8], j = 8*p + t.
        # Fully contiguous in DRAM, 256B per partition.
        stg = sb.tile([P, T, 8], f32)
        nc.sync.dma_start(stg[:], d_all.ap().rearrange("(p t) c -> p t c", p=P))

        t_pred = stg[:, :, 0:3]
        t_true = stg[:, :, 3:6]
        t_mask = stg[:, :, 6]

        def body():
            d = wpool.tile([P, T, 3], f32, tag="d", name="d")
            nc.vector.tensor_tensor(d[:], t_pred, t_true, Alu.subtract)
            sq = wpool.tile([P, T, 3], f32, tag="sq", name="sq")
            nc.gpsimd.tensor_tensor(sq[:], d[:], d[:], Alu.mult)
            nsq = wpool.tile([P, T], f32, tag="nsq", name="nsq")
            nc.vector.tensor_reduce(nsq[:], sq[:], AxX, Alu.add)
            nsqm = wpool.tile([P, T], f32, tag="nsqm", name="nsqm")
            nc.vector.tensor_tensor(nsqm[:], nsq[:], t_mask, Alu.mult)

            dm = wpool.tile([P, T], f32, tag="dm", name="dm")
            nc.scalar.activation(dm[:], nsqm[:], Act.Sqrt)
            s = wpool.tile([P, T], f32, tag="s", name="s")
            nc.vector.scalar_tensor_tensor(s[:], dm[:], 10.0, dm[:],
                                           Alu.min, Alu.add)

            ps = pspool.tile([1, T], f32, tag="ps", name="ps")
            nc.tensor.matmul(ps[:], ones[:], s[:], start=True, stop=True)
            sbo = opool.tile([1, 1], f32, tag="sbo", name="sbo")
            nc.vector.tensor_reduce(sbo[:], ps[:], AxX, Alu.add)
            nc.gpsimd.dma_start(d_out.ap(), sbo[:])

        if reps:
            assert reps % UNROLL == 0, f"reps must be a multiple of {UNROLL}"
            with tc.For_i(0, reps // UNROLL, 1):
                for _ in range(UNROLL):
                    body()
        else:
            body()

    nc.compile()
    return nc


def _get_nc():
    if "nc" not in _cache:
        _cache["nc"] = _build_nc()
    return _cache["nc"]


def make_inmaps(n, ca, c, pred_pos, true_pos, mask):
    allc = np.empty((B, N, 8), np.float32)
    allc[:, :, 0:3] = np.asarray(pred_pos, np.float32)
    allc[:, :, 3:6] = np.asarray(true_pos, np.float32)
    allc[:, :, 6] = np.asarray(mask).astype(np.float32)
    allc[:, :, 7] = 0.0
    return [{"all_in": allc[b]} for b in range(B)]


def kernel(n, ca, c, pred_pos, true_pos, mask) -> np.ndarray:
    from concourse.bass_utils import run_bass_kernel_spmd

    nc = _get_nc()
    in_maps = make_inmaps(n, ca, c, pred_pos, true_pos, mask)
    res = run_bass_kernel_spmd(nc, in_maps, core_ids=list(range(NCORES)))
    m = np.asarray(mask).astype(np.float64)
    c_b = m.sum(axis=1)                      # per-batch masked-residue count
    total = 0.0
    for b in range(B):
        sheet = res.results[b]["out_acc"].astype(np.float64)
        total += c_b[b] * 0.5 * sheet.sum()
    return np.asarray(total / (m.sum() + 1e-8), dtype=np.float32)


# revision 9
# speedup vs baseline: 15.0145x; 3.0689x over previous
"""FAPE loss kernel for Trainium2 (Bass/Tile), 8 NeuronCores.

Problem: B=8, N=1024.  reference computes, per batch b:
    R_i, t_i = backbone frames from (n, ca, c)          [N,3,3],[N,3]
    diff[i,j] = || R_i^T (pred_j - t_i) - R_i^T (true_j - t_i) ||
    per_pair  = min(diff,10) + 0.5*(diff - min(diff,10)) = 0.5*(diff + min(diff,10))
    out = sum_b sum_ij m_i m_j per_pair / (sum(m) + 1e-8)

Key identity: both pred and true are expressed in the SAME frame i, so
    R_i^T (pred_j - t_i) - R_i^T (true_j - t_i) = R_i^T d_j,  d_j = pred_j - true_j
and R_i is orthonormal by construction (x, y, z mutually orthogonal unit
vectors from normalized cross products), hence
    diff[i,j] = ||R_i^T d_j|| = ||d_j||            (independent of i!)
up to the 1e-8 normalize-eps and f32 rounding (~1e-7 relative, verified
6e-7 end-to-end vs the jax reference; tolerance is 2e-2).  The O(N^2)
pairwise reduction therefore factorizes exactly:
    sum_ij m_i m_j f(||d_j||) = (sum_i m_i) * (sum_j m_j f(||d_j||))
leaving O(N) device work per batch: one masked norm + clamp + row-sum.

Per-core body (one batch per core, j = 8*p + t, 9 instructions).  The
host only needs the single scalar sum_j m_j*(diff_j + min(diff_j,10)),
so clamp+add fuse into one scalar_tensor_tensor and the whole thing
funnels through one PE partition-reduce:
    d    = pred - true                       [128,8,3]  DVE
    sq   = d*d                               [128,8,3]  Pool
    nsq  = reduce_X(sq)                      [128,8]    DVE
    nsqm = nsq * mask   (m in {0,1}: sqrt(m*nsq) = m*diff)       DVE
    dm   = sqrt(nsqm)           (= m*diff)   [128,8]    ACT
    s    = (dm min 10) + dm                  [128,8]    DVE
    ps   = ones[128,1]^T @ s[128,8] -> [1,8]  (partition sum)    PE
    sbo  = reduce_X(ps) -> [1,1]                                 DVE
    DMA sbo [1,1] -> HBM  (single 4-byte descriptor, Pool-queue
    issued: DMA_SEQ_TIME is ~25ns on Pool vs ~565ns on SP)
Host scales by 0.5 and the (sum_i m_i) factor per batch (f64), and
normalizes by sum(m) + 1e-8.

The bench loop (reps>0) unrolls UNROLL bodies per For_i iteration: the
For_i back-edge runs an all-engine barrier (~1.3us) that would otherwise
dwarf the body; unrolling amortizes it and lets consecutive bodies
pipeline through the tile-pool buffer rotation, so the measured slope is
the true steady-state per-body cost.

Sharding: batch-parallel, one batch per core (spec hint allows B data-parallel).
"""

import numpy as np

P = 128          # partitions
T = 8            # j = 8*p + t  (p-major; any index bijection works for the sum)
N = 1024
B = 8
NCORES = 8
UNROLL = 20

_cache: dict = {}


def _build_nc(reps=0, prep_only=False):
    """Emit the single-core BIR module (same NEFF runs SPMD on all 8 cores)."""
    from contextlib import ExitStack

    import concourse.bacc as bacc
    import concourse.mybir as mybir
    import concourse.tile as tile
    from concourse._compat import axon_active

    f32 = mybir.dt.float32
    Alu = mybir.AluOpType
    Act = mybir.ActivationFunctionType
    AxX = mybir.AxisListType.X

    nc = bacc.Bacc(
        "TRN2",
        target_bir_lowering=False,
        debug=not axon_active(),
        num_devices=NCORES,
    )

    # One concatenated input: cols [pred(3) true(3) mask(1) pad(1)]
    d_all = nc.dram_tensor("all_in", [N, 8], f32, kind="ExternalInput")
    # UNROLL rows so unrolled bench bodies write distinct addresses (no
    # artificial WAW chain between their DMAs); the real kernel and the
    # host only use row 0.
    d_out = nc.dram_tensor("out_acc", [UNROLL, 1], f32, kind="ExternalOutput")

    with tile.TileContext(nc) as tc, ExitStack() as ctx:
        sb = ctx.enter_context(tc.tile_pool(name="sb", bufs=1))
        wpool = ctx.enter_context(tc.tile_pool(name="wpool", bufs=6))
        opool = ctx.enter_context(tc.tile_pool(name="opool", bufs=6))
        pspool = ctx.enter_context(tc.tile_pool(name="pspool", bufs=8,
                                                space="PSUM"))

        # ---- ACT table warmup: force the sqrt set load early (overlaps DMA)
        warm = sb.tile([1, 2], f32)
        nc.vector.memset(warm[:], 1.0)
        nc.scalar.activation(warm[:, 1:2], warm[:, 0:1], Act.Sqrt)

        ones = sb.tile([P, 1], f32)
        nc.vector.memset(ones[:], 1.0)

        # ---- ONE input DMA: [1024,8] -> [128, 8, 8], j = 8*p + t.
        # Fully contiguous in DRAM, 256B per partition.
        stg = sb.tile([P, T, 8], f32)
        nc.sync.dma_start(stg[:], d_all.ap().rearrange("(p t) c -> p t c", p=P))

        t_pred = stg[:, :, 0:3]
        t_true = stg[:, :, 3:6]
        t_mask = stg[:, :, 6]

        def body(row=0):
            d = wpool.tile([P, T, 3], f32, tag="d", name="d")
            nc.vector.tensor_tensor(d[:], t_pred, t_true, Alu.subtract)
            sq = wpool.tile([P, T, 3], f32, tag="sq", name="sq")
            nc.gpsimd.tensor_tensor(sq[:], d[:], d[:], Alu.mult)
            nsq = wpool.tile([P, T], f32, tag="nsq", name="nsq")
            nc.vector.tensor_reduce(nsq[:], sq[:], AxX, Alu.add)
            nsqm = wpool.tile([P, T], f32, tag="nsqm", name="nsqm")
            nc.vector.tensor_tensor(nsqm[:], nsq[:], t_mask, Alu.mult)

            dm = wpool.tile([P, T], f32, tag="dm", name="dm")
            nc.scalar.activation(dm[:], nsqm[:], Act.Sqrt)
            s = wpool.tile([P, T], f32, tag="s", name="s")
            nc.vector.scalar_tensor_tensor(s[:], dm[:], 10.0, dm[:],
                                           Alu.min, Alu.add)

            ps = pspool.tile([1, T], f32, tag="ps", name="ps")
            nc.tensor.matmul(ps[:], ones[:], s[:], start=True, stop=True)
            sbo = opool.tile([1, 1], f32, tag="sbo", name="sbo")
            nc.vector.tensor_reduce(sbo[:], ps[:], AxX, Alu.add)
            nc.gpsimd.dma_start(d_out.ap()[row:row + 1, :], sbo[:])

        if reps:
            assert reps % UNROLL == 0, f"reps must be a multiple of {UNROLL}"
            with tc.For_i(0, reps // UNROLL, 1):
                for u in range(UNROLL):
                    body(u)
        else:
            body()

    nc.compile()
    return nc


def _get_nc():
    if "nc" not in _cache:
        _cache["nc"] = _build_nc()
    return _cache["nc"]


def make_inmaps(n, ca, c, pred_pos, true_pos, mask):
    allc = np.empty((B, N, 8), np.float32)
    allc[:, :, 0:3] = np.asarray(pred_pos, np.float32)
    allc[:, :, 3:6] = np.asarray(true_pos, np.float32)
    allc[:, :, 6] = np.asarray(mask).astype(np.float32)
    allc[:, :, 7] = 0.0
    return [{"all_in": allc[b]} for b in range(B)]


def kernel(n, ca, c, pred_pos, true_pos, mask) -> np.ndarray:
    from concourse.bass_utils import run_bass_kernel_spmd

    nc = _get_nc()
    in_maps = make_inmaps(n, ca, c, pred_pos, true_pos, mask)
    res = run_bass_kernel_spmd(nc, in_maps, core_ids=list(range(NCORES)))
    m = np.asarray(mask).astype(np.float64)
    c_b = m.sum(axis=1)                      # per-batch masked-residue count
    total = 0.0
    for b in range(B):
        sheet = res.results[b]["out_acc"][0].astype(np.float64)
        total += c_b[b] * 0.5 * sheet.sum()
    return np.asarray(total / (m.sum() + 1e-8), dtype=np.float32)


# revision 10
# speedup vs baseline: 42.7624x; 2.8481x over previous
"""FAPE loss kernel for Trainium2 (Bass/Tile), 8 NeuronCores.

Problem: B=8, N=1024.  reference computes, per batch b:
    R_i, t_i = backbone frames from (n, ca, c)          [N,3,3],[N,3]
    diff[i,j] = || R_i^T (pred_j - t_i) - R_i^T (true_j - t_i) ||
    per_pair  = min(diff,10) + 0.5*(diff - min(diff,10)) = 0.5*(diff + min(diff,10))
    out = sum_b sum_ij m_i m_j per_pair / (sum(m) + 1e-8)

Key identity: both pred and true are expressed in the SAME frame i, so
    R_i^T (pred_j - t_i) - R_i^T (true_j - t_i) = R_i^T d_j,  d_j = pred_j - true_j
and R_i is orthonormal by construction (x, y, z mutually orthogonal unit
vectors from normalized cross products), hence
    diff[i,j] = ||R_i^T d_j|| = ||d_j||            (independent of i!)
up to the 1e-8 normalize-eps and f32 rounding (~1e-7 relative, verified
6e-7 end-to-end vs the jax reference; tolerance is 2e-2).  The O(N^2)
pairwise reduction therefore factorizes exactly:
    sum_ij m_i m_j f(||d_j||) = (sum_i m_i) * (sum_j m_j f(||d_j||))
leaving O(N) device work per batch: one masked norm + clamp + row-sum.

Per-core body (one batch per core, j = 8*p + t, 9 instructions).  The
host only needs the single scalar sum_j m_j*(diff_j + min(diff_j,10)),
so clamp+add fuse into one scalar_tensor_tensor and the whole thing
funnels through one PE partition-reduce:
    d    = pred - true                       [128,8,3]  DVE
    sq   = d*d                               [128,8,3]  Pool
    nsq  = reduce_X(sq)                      [128,8]    DVE
    nsqm = nsq * mask   (m in {0,1}: sqrt(m*nsq) = m*diff)       DVE
    dm   = sqrt(nsqm)           (= m*diff)   [128,8]    ACT
    s    = (dm min 10) + dm                  [128,8]    DVE
    ps   = ones[128,1]^T @ s[128,8] -> [1,8]  (partition sum)    PE
    sbo  = reduce_X(ps) -> [1,1]                                 DVE
    DMA sbo [1,1] -> HBM  (single 4-byte descriptor, SP-issued HWDGE;
    Pool-issued DMA is SWDGE whose ~1us descriptor generation runs on
    the Pool engine itself)
Host scales by 0.5 and the (sum_i m_i) factor per batch (f64), and
normalizes by sum(m) + 1e-8.

The bench loop (reps>0) unrolls UNROLL bodies per For_i iteration: the
For_i back-edge runs an all-engine barrier (~1.3us) that would otherwise
dwarf the body; unrolling amortizes it and lets consecutive bodies
pipeline through the tile-pool buffer rotation, so the measured slope is
the true steady-state per-body cost.

Sharding: batch-parallel, one batch per core (spec hint allows B data-parallel).
"""

import numpy as np

P = 128          # partitions
T = 8            # j = 8*p + t  (p-major; any index bijection works for the sum)
N = 1024
B = 8
NCORES = 8
UNROLL = 20

_cache: dict = {}


def _build_nc(reps=0, prep_only=False):
    """Emit the single-core BIR module (same NEFF runs SPMD on all 8 cores)."""
    from contextlib import ExitStack

    import concourse.bacc as bacc
    import concourse.mybir as mybir
    import concourse.tile as tile
    from concourse._compat import axon_active

    f32 = mybir.dt.float32
    Alu = mybir.AluOpType
    Act = mybir.ActivationFunctionType
    AxX = mybir.AxisListType.X

    nc = bacc.Bacc(
        "TRN2",
        target_bir_lowering=False,
        debug=not axon_active(),
        num_devices=NCORES,
    )

    # One concatenated input: cols [pred(3) true(3) mask(1) pad(1)]
    d_all = nc.dram_tensor("all_in", [N, 8], f32, kind="ExternalInput")
    # UNROLL rows so unrolled bench bodies write distinct addresses (no
    # artificial WAW chain between their DMAs); the real kernel and the
    # host only use row 0.
    d_out = nc.dram_tensor("out_acc", [UNROLL, 1], f32, kind="ExternalOutput")

    with tile.TileContext(nc) as tc, ExitStack() as ctx:
        sb = ctx.enter_context(tc.tile_pool(name="sb", bufs=1))
        wpool = ctx.enter_context(tc.tile_pool(name="wpool", bufs=6))
        opool = ctx.enter_context(tc.tile_pool(name="opool", bufs=6))
        pspool = ctx.enter_context(tc.tile_pool(name="pspool", bufs=8,
                                                space="PSUM"))

        # ---- ACT table warmup: force the sqrt set load early (overlaps DMA)
        warm = sb.tile([1, 2], f32)
        nc.vector.memset(warm[:], 1.0)
        nc.scalar.activation(warm[:, 1:2], warm[:, 0:1], Act.Sqrt)

        ones = sb.tile([P, 1], f32)
        nc.vector.memset(ones[:], 1.0)

        # ---- ONE input DMA: [1024,8] -> [128, 8, 8], j = 8*p + t.
        # Fully contiguous in DRAM, 256B per partition.
        stg = sb.tile([P, T, 8], f32)
        nc.sync.dma_start(stg[:], d_all.ap().rearrange("(p t) c -> p t c", p=P))

        t_pred = stg[:, :, 0:3]
        t_true = stg[:, :, 3:6]
        t_mask = stg[:, :, 6]

        def body(row=0):
            d = wpool.tile([P, T, 3], f32, tag="d", name="d")
            nc.vector.tensor_tensor(d[:], t_pred, t_true, Alu.subtract)
            sq = wpool.tile([P, T, 3], f32, tag="sq", name="sq")
            nc.gpsimd.tensor_tensor(sq[:], d[:], d[:], Alu.mult)
            nsq = wpool.tile([P, T], f32, tag="nsq", name="nsq")
            nc.vector.tensor_reduce(nsq[:], sq[:], AxX, Alu.add)
            nsqm = wpool.tile([P, T], f32, tag="nsqm", name="nsqm")
            nc.vector.tensor_tensor(nsqm[:], nsq[:], t_mask, Alu.mult)

            dm = wpool.tile([P, T], f32, tag="dm", name="dm")
            nc.scalar.activation(dm[:], nsqm[:], Act.Sqrt)
            s = wpool.tile([P, T], f32, tag="s", name="s")
            nc.vector.scalar_tensor_tensor(s[:], dm[:], 10.0, dm[:],
                                           Alu.min, Alu.add)

            ps = pspool.tile([1, T], f32, tag="ps", name="ps")
            nc.tensor.matmul(ps[:], ones[:], s[:], start=True, stop=True)
            sbo = opool.tile([1, 1], f32, tag="sbo", name="sbo")
            nc.vector.tensor_reduce(sbo[:], ps[:], AxX, Alu.add)
            nc.sync.dma_start(d_out.ap()[row:row + 1, :], sbo[:])

        if reps:
            assert reps % UNROLL == 0, f"reps must be a multiple of {UNROLL}"
            with tc.For_i(0, reps // UNROLL, 1):
                for u in range(UNROLL):
                    body(u)
        else:
            body()

    nc.compile()
    return nc


def _get_nc():
    if "nc" not in _cache:
        _cache["nc"] = _build_nc()
    return _cache["nc"]


def make_inmaps(n, ca, c, pred_pos, true_pos, mask):
    allc = np.empty((B, N, 8), np.float32)
    allc[:, :, 0:3] = np.asarray(pred_pos, np.float32)
    allc[:, :, 3:6] = np.asarray(true_pos, np.float32)
    allc[:, :, 6] = np.asarray(mask).astype(np.float32)
    allc[:, :, 7] = 0.0
    return [{"all_in": allc[b]} for b in range(B)]


def kernel(n, ca, c, pred_pos, true_pos, mask) -> np.ndarray:
    from concourse.bass_utils import run_bass_kernel_spmd

    nc = _get_nc()
    in_maps = make_inmaps(n, ca, c, pred_pos, true_pos, mask)
    res = run_bass_kernel_spmd(nc, in_maps, core_ids=list(range(NCORES)))
    m = np.asarray(mask).astype(np.float64)
    c_b = m.sum(axis=1)                      # per-batch masked-residue count
    total = 0.0
    for b in range(B):
        sheet = res.results[b]["out_acc"][0].astype(np.float64)
        total += c_b[b] * 0.5 * sheet.sum()
    return np.asarray(total / (m.sum() + 1e-8), dtype=np.float32)
